# revision 2
# baseline (speedup 1.0000x reference)
"""Trainium2 Bass kernel for the dual-graph GCN + exp-cross-attention problem.

Reference computation (N=4096, DIM_IN=256, DIM_H=DIM_OUT=128):
    Xs = relu((A_s @ X_s) @ W1); Xs = relu((A_s @ Xs) @ W2)
    Xt = relu((A_t @ X_t) @ W1); Xt = relu((A_t @ Xt) @ W2)
    S  = exp((Xs @ W3) @ Xt.T)
    out_s = (S @ Xs) @ W4; out_t = (S @ Xt) @ W4
    return (out_s, out_t, S)

Sharding: node dim N row-sharded across 8 cores (512 rows each). Each core
holds its A row-shards (staged transposed so the contraction dim is on
partitions) resident in SBUF for both GCN layers. All intermediate
activations are kept feature-major ([128 feat, nodes]) so no on-device
transposes are needed; the host transposes at staging/gather time. Two
in-kernel AllGathers exchange the (tiny) per-core layer outputs between
GCN layers. Compute dtype bf16 with fp32 PSUM accumulation.
"""

import sys

for _p in ("/opt/trn_rl_repo",):
    if _p not in sys.path:
        sys.path.insert(0, _p)

import numpy as np
import ml_dtypes

import concourse.bass as bass
import concourse.mybir as mybir
import concourse.tile as tile
from concourse import bacc
from concourse.bass_utils import run_bass_kernel_spmd

N = 4096
F_IN = 256
F = 128
NCORES = 8
RPC = N // NCORES  # 512 rows per core
P = 128
CH = N // P  # 32 contraction chunks over the node dim
BPC = RPC // P  # 4 node-chunks per rank block

CDT = mybir.dt.bfloat16
NP_CDT = ml_dtypes.bfloat16
F32 = mybir.dt.float32

_compiled = None


def _build():
    nc = bacc.Bacc("TRN2", target_bir_lowering=False, debug=False,
                   num_devices=NCORES)

    # Per-core inputs. a*t = A[rows_k, :].T -> [N, RPC] (source nodes x rows).
    ast_d = nc.dram_tensor("ast", [N, RPC], CDT, kind="ExternalInput")
    att_d = nc.dram_tensor("att", [N, RPC], CDT, kind="ExternalInput")
    # Full features, transposed: [F_IN, N].
    xst_d = nc.dram_tensor("xst", [F_IN, N], CDT, kind="ExternalInput")
    xtt_d = nc.dram_tensor("xtt", [F_IN, N], CDT, kind="ExternalInput")
    w1_d = nc.dram_tensor("w1", [F_IN, F], CDT, kind="ExternalInput")
    w2_d = nc.dram_tensor("w2", [F, F], CDT, kind="ExternalInput")
    w3_d = nc.dram_tensor("w3", [F, F], CDT, kind="ExternalInput")
    w4_d = nc.dram_tensor("w4", [F, F], CDT, kind="ExternalInput")

    # Per-core outputs: S.T shard [N kv, RPC q]; out_{s,t}.T [F, RPC q].
    st_d = nc.dram_tensor("st_out", [N, RPC], CDT, kind="ExternalOutput")
    os_d = nc.dram_tensor("outs_t", [F, RPC], F32, kind="ExternalOutput")
    ot_d = nc.dram_tensor("outt_t", [F, RPC], F32, kind="ExternalOutput")

    rg = [list(range(NCORES))]

    with tile.TileContext(nc) as tc:
        with (
            tc.tile_pool(name="wpool", bufs=1) as wpool,
            tc.tile_pool(name="xpool", bufs=1) as xpool,
            tc.tile_pool(name="apool", bufs=1) as apool,
            tc.tile_pool(name="actp", bufs=1) as actp,
            tc.tile_pool(name="spool", bufs=4) as spool,
            tc.tile_pool(name="pk", bufs=2, space="PSUM") as pk,
            tc.tile_pool(name="pacc", bufs=2, space="PSUM") as pacc,
            tc.tile_pool(name="pout", bufs=2, space="PSUM") as pout,
            tc.tile_pool(name="psp", bufs=2, space="PSUM") as psp,
            tc.tile_pool(name="dram", bufs=1, space="DRAM") as dram,
        ):
            # ---- weights ----
            w1_sb = wpool.tile([P, 2, F], CDT)
            nc.sync.dma_start(w1_sb[:], w1_d[:].rearrange("(c p) f -> p c f", p=P))
            w2_sb = wpool.tile([P, F], CDT)
            nc.sync.dma_start(w2_sb[:], w2_d[:])
            w3_sb = wpool.tile([P, F], CDT)
            nc.sync.dma_start(w3_sb[:], w3_d[:])
            w4_sb = wpool.tile([P, F], CDT)
            nc.sync.dma_start(w4_sb[:], w4_d[:])

            # ---- X.T full, both graphs: [P, 2, N] ----
            x_sb = {}
            for g, xd in (("s", xst_d), ("t", xtt_d)):
                x_sb[g] = xpool.tile([P, 2, N], CDT, name=f"x_{g}")
                nc.sync.dma_start(x_sb[g][:],
                                  xd[:].rearrange("(c p) n -> p c n", p=P))

            # ---- A shards resident in SBUF: [P, CH, RPC] ----
            a_sb = {}
            for g, ad in (("s", ast_d), ("t", att_d)):
                a_sb[g] = apool.tile([P, CH, RPC], CDT, name=f"a_{g}")
                a_src = ad[:].rearrange("(c p) q -> p c q", p=P)
                for i in range(4):
                    nc.sync.dma_start(a_sb[g][:, i * 8:(i + 1) * 8, :],
                                      a_src[:, i * 8:(i + 1) * 8, :])

            # ---- Y1 = X @ W1, full N, node-major tiles [P, CH, F] ----
            y1_sb = {}
            for g in ("s", "t"):
                y1_sb[g] = actp.tile([P, CH, F], CDT, name=f"y1_{g}")
                for i in range(CH // 4):
                    ps = pk.tile([P, 4, F], F32, name="y1ps", tag="pack")
                    for j in range(4):
                        c = 4 * i + j
                        for ci in range(2):
                            nc.tensor.matmul(
                                ps[:, j, :],
                                lhsT=x_sb[g][:, ci, c * P:(c + 1) * P],
                                rhs=w1_sb[:, ci, :],
                                start=(ci == 0), stop=(ci == 1),
                            )
                    nc.vector.tensor_copy(y1_sb[g][:, 4 * i:4 * i + 4, :], ps[:])

            # ---- layer 1: X1_k.T = relu((A_k @ Y1).T) : [P feat, RPC] ----
            cc1_in = dram.tile([P, 2 * RPC], CDT)
            cc1_out = dram.tile([NCORES * P, 2 * RPC], CDT, addr_space="Shared")
            x1T_sb = {}
            for gi, g in enumerate(("s", "t")):
                t1 = pacc.tile([P, RPC], F32, name=f"t1_{g}", tag="acc")
                for c in range(CH):
                    nc.tensor.matmul(t1[:], lhsT=y1_sb[g][:, c, :],
                                     rhs=a_sb[g][:, c, :],
                                     start=(c == 0), stop=(c == CH - 1))
                x1T_sb[g] = actp.tile([P, RPC], CDT, name=f"x1T_{g}")
                nc.scalar.activation(x1T_sb[g][:], t1[:],
                                     mybir.ActivationFunctionType.Relu)
                nc.sync.dma_start(cc1_in[:, gi * RPC:(gi + 1) * RPC],
                                  x1T_sb[g][:])

            nc.gpsimd.collective_compute(
                "AllGather", mybir.AluOpType.bypass, replica_groups=rg,
                ins=[cc1_in[:].opt()], outs=[cc1_out[:].opt()],
            )

            # gathered layer-1 activations, feature-major blocks [P, NCORES, RPC]
            x1all = {}
            for gi, g in enumerate(("s", "t")):
                x1all[g] = actp.tile([P, NCORES, RPC], CDT, name=f"x1all_{g}")
                src = cc1_out[:, gi * RPC:(gi + 1) * RPC]
                nc.sync.dma_start(x1all[g][:],
                                  src.rearrange("(b p) q -> p b q", p=P))

            # ---- Z = X1 @ W2, node-major tiles [P, CH, F] ----
            z_sb = {}
            for g in ("s", "t"):
                z_sb[g] = actp.tile([P, CH, F], CDT, name=f"z_{g}")
                for i in range(CH // 4):
                    ps = pk.tile([P, 4, F], F32, name="zps", tag="pack")
                    for j in range(4):
                        c = 4 * i + j
                        b, o = c // BPC, (c % BPC) * P
                        nc.tensor.matmul(ps[:, j, :],
                                         lhsT=x1all[g][:, b, o:o + P],
                                         rhs=w2_sb[:],
                                         start=True, stop=True)
                    nc.vector.tensor_copy(z_sb[g][:, 4 * i:4 * i + 4, :], ps[:])

            # ---- layer 2: X2_k.T = relu((A_k @ Z).T) ----
            cc2_in = dram.tile([P, 2 * RPC], CDT)
            cc2_out = dram.tile([NCORES * P, 2 * RPC], CDT, addr_space="Shared")
            x2T_sb = {}
            for gi, g in enumerate(("s", "t")):
                t2 = pacc.tile([P, RPC], F32, name=f"t2_{g}", tag="acc")
                for c in range(CH):
                    nc.tensor.matmul(t2[:], lhsT=z_sb[g][:, c, :],
                                     rhs=a_sb[g][:, c, :],
                                     start=(c == 0), stop=(c == CH - 1))
                x2T_sb[g] = actp.tile([P, RPC], CDT, name=f"x2T_{g}")
                nc.scalar.activation(x2T_sb[g][:], t2[:],
                                     mybir.ActivationFunctionType.Relu)
                nc.sync.dma_start(cc2_in[:, gi * RPC:(gi + 1) * RPC],
                                  x2T_sb[g][:])

            nc.gpsimd.collective_compute(
                "AllGather", mybir.AluOpType.bypass, replica_groups=rg,
                ins=[cc2_in[:].opt()], outs=[cc2_out[:].opt()],
            )

            x2all = {}
            for gi, g in enumerate(("s", "t")):
                x2all[g] = actp.tile([P, NCORES, RPC], CDT, name=f"x2all_{g}")
                src = cc2_out[:, gi * RPC:(gi + 1) * RPC]
                nc.sync.dma_start(x2all[g][:],
                                  src.rearrange("(b p) q -> p b q", p=P))

            # ---- Q.T = W3.T @ X2s_k.T : [P f, RPC q] (local rows only) ----
            qps = pk.tile([P, RPC], F32, name="qps", tag="pack")
            nc.tensor.matmul(qps[:], lhsT=w3_sb[:], rhs=x2T_sb["s"][:],
                             start=True, stop=True)
            q_sb = actp.tile([P, RPC], CDT)
            nc.vector.tensor_copy(q_sb[:], qps[:])

            # ---- G = X2 @ W4, node-major tiles [P, CH, F], both graphs ----
            g_sb = {}
            for g in ("s", "t"):
                g_sb[g] = actp.tile([P, CH, F], CDT, name=f"g_{g}")
                for i in range(CH // 4):
                    ps = pk.tile([P, 4, F], F32, name="gps", tag="pack")
                    for j in range(4):
                        c = 4 * i + j
                        b, o = c // BPC, (c % BPC) * P
                        nc.tensor.matmul(ps[:, j, :],
                                         lhsT=x2all[g][:, b, o:o + P],
                                         rhs=w4_sb[:],
                                         start=True, stop=True)
                    nc.vector.tensor_copy(g_sb[g][:, 4 * i:4 * i + 4, :], ps[:])

            # ---- attention: S.T chunks, exp, store, accumulate outputs ----
            ops_ = pout.tile([P, RPC], F32, name="ops", tag="out")
            opt_ = pout.tile([P, RPC], F32, name="opt", tag="out")
            for c in range(CH):
                b, o = c // BPC, (c % BPC) * P
                sps = psp.tile([P, RPC], F32, name="sps", tag="sps")
                nc.tensor.matmul(sps[:], lhsT=x2all["t"][:, b, o:o + P],
                                 rhs=q_sb[:], start=True, stop=True)
                s_sb = spool.tile([P, RPC], CDT, name="s_sb")
                nc.scalar.activation(s_sb[:], sps[:],
                                     mybir.ActivationFunctionType.Exp)
                nc.sync.dma_start(st_d[c * P:(c + 1) * P, :], s_sb[:])
                nc.tensor.matmul(ops_[:], lhsT=g_sb["s"][:, c, :], rhs=s_sb[:],
                                 start=(c == 0), stop=(c == CH - 1))
                nc.tensor.matmul(opt_[:], lhsT=g_sb["t"][:, c, :], rhs=s_sb[:],
                                 start=(c == 0), stop=(c == CH - 1))

            os_sb = actp.tile([P, RPC], F32)
            nc.vector.tensor_copy(os_sb[:], ops_[:])
            nc.sync.dma_start(os_d[:], os_sb[:])
            ot_sb = actp.tile([P, RPC], F32)
            nc.vector.tensor_copy(ot_sb[:], opt_[:])
            nc.sync.dma_start(ot_d[:], ot_sb[:])

    nc.compile()
    return nc


def _get_compiled():
    global _compiled
    if _compiled is None:
        _compiled = _build()
    return _compiled


def _prep_in_maps(inputs):
    A_s, X_s = inputs["A_s"], inputs["X_s"]
    A_t, X_t = inputs["A_t"], inputs["X_t"]

    def c(x):
        return np.ascontiguousarray(x).astype(NP_CDT)

    shared = {
        "xst": c(X_s.T), "xtt": c(X_t.T),
        "w1": c(inputs["W1"]), "w2": c(inputs["W2"]),
        "w3": c(inputs["W3"]), "w4": c(inputs["W4"]),
    }
    in_maps = []
    for k in range(NCORES):
        rows = slice(k * RPC, (k + 1) * RPC)
        in_maps.append({
            "ast": c(A_s[rows, :].T),
            "att": c(A_t[rows, :].T),
            **shared,
        })
    return in_maps


def _run(inputs, trace=False, **kwargs):
    nc = _get_compiled()
    in_maps = _prep_in_maps(inputs)
    res = run_bass_kernel_spmd(nc, in_maps, core_ids=list(range(NCORES)),
                               trace=trace, **kwargs)

    S = np.empty((N, N), dtype=np.float32)
    out_s = np.empty((N, F), dtype=np.float32)
    out_t = np.empty((N, F), dtype=np.float32)
    for k in range(NCORES):
        r = res.results[k]
        rows = slice(k * RPC, (k + 1) * RPC)
        S[rows, :] = r["st_out"].astype(np.float32).T
        out_s[rows, :] = r["outs_t"].T
        out_t[rows, :] = r["outt_t"].T
    return (out_s, out_t, S), res


def kernel(**inputs):
    outputs, _ = _run(inputs, trace=False)
    return outputs
